# revision 4
# baseline (speedup 1.0000x reference)
"""MaxPool1d(K=4, stride=2, VALID) over ragged NaN-padded sequences.

Full input:  x  [16, 8, 64, 8192] f32, channel c valid prefix LENS[c], NaN tail.
Full output:    [16, 8, 64, 4095] f32, pooled valid prefix, NaN tail.

Sharding: data-parallel over batch — 16 batches / 8 cores = 2 per core.
Per core, for each channel c the 2 batches x 64 features form exactly 128
partition rows of length LENS[c]; pooling runs along the free dim:
  stage 1: m[i]   = max(x[2i], x[2i+1])      (stride-2 tensor_tensor max)
  stage 2: out[p] = max(m[p], m[p+1])        (unit-stride tensor_tensor max)
Only the valid prefix is ever read from HBM; output NaN tails come from an
SBUF memset, never from DRAM reads.
"""

import numpy as np

# ---- problem constants (hardcoded; kernel.py must be self-contained) ----
B, C, F, S = 16, 8, 64, 8192
K, STRIDE = 4, 2
P_OUT = (S - K) // STRIDE + 1  # 4095
LENS = [8192, 4096, 2048, 1024, 8192, 4096, 2048, 1024]
N_CORES = 8
B_LOC = B // N_CORES  # 2 batches per core

_CACHE = {}


def _build_nc():
    import concourse.bacc as bacc
    import concourse.mybir as mybir
    from concourse.tile import TileContext

    nc = bacc.Bacc("TRN2", debug=False, num_devices=N_CORES)
    x = nc.declare_dram_parameter(
        "x", [B_LOC, C, F, S], mybir.dt.float32, isOutput=False
    )
    out = nc.declare_dram_parameter(
        "out", [B_LOC, C, F, P_OUT], mybir.dt.float32, isOutput=True
    )
    x_ap = x.ap()
    out_ap = out.ap()

    # Big channels first so the DMA pipeline fills early.
    order = [0, 4, 1, 5, 2, 6, 3, 7]

    with TileContext(nc) as tc:
        with tc.tile_pool(name="xin", bufs=2) as in_pool, \
             tc.tile_pool(name="mid", bufs=2) as mid_pool, \
             tc.tile_pool(name="res", bufs=2) as out_pool:
            for c in order:
                L = LENS[c]
                Pv = (L - K) // STRIDE + 1

                xin = in_pool.tile([128, L], mybir.dt.float32, tag="xin")
                # [2, 64, L] valid prefix -> 128 partitions (flattened order)
                nc.sync.dma_start(out=xin[:], in_=x_ap[:, c, :, 0:L])

                m = mid_pool.tile([128, L // 2], mybir.dt.float32, tag="mid")
                x2 = xin[:].rearrange("p (n two) -> p n two", two=2)
                nc.vector.tensor_max(m[:], x2[:, :, 0], x2[:, :, 1])

                o = out_pool.tile([128, P_OUT], mybir.dt.float32, tag="res")
                nc.vector.tensor_max(o[:, 0:Pv], m[:, 0:Pv], m[:, 1:Pv + 1])
                if Pv < P_OUT:
                    nc.vector.memset(o[:, Pv:P_OUT], float("nan"))

                # store on the ACT HWDGE ring so loads/stores overlap
                nc.scalar.dma_start(out=out_ap[:, c, :, :], in_=o[:])
    nc.compile()
    return nc


def _get_nc():
    if "nc" not in _CACHE:
        _CACHE["nc"] = _build_nc()
    return _CACHE["nc"]


def kernel(x: np.ndarray) -> np.ndarray:
    from concourse.bass_utils import run_bass_kernel_spmd

    x = np.asarray(x, dtype=np.float32)
    assert x.shape == (B, C, F, S), x.shape

    nc = _get_nc()
    in_maps = [
        {"x": np.ascontiguousarray(x[i * B_LOC:(i + 1) * B_LOC])}
        for i in range(N_CORES)
    ]
    res = run_bass_kernel_spmd(nc, in_maps, list(range(N_CORES)))
    return np.concatenate([r["out"] for r in res.results], axis=0)


# revision 7
# speedup vs baseline: 1.9620x; 1.9620x over previous
"""MaxPool1d(K=4, stride=2, VALID) over ragged NaN-padded sequences.

Full input:  x  [16, 8, 64, 8192] f32, channel c valid prefix LENS[c], NaN tail.
Full output:    [16, 8, 64, 4095] f32, pooled valid prefix, NaN tail.

Sharding: data-parallel over batch — 16 batches / 8 cores = 2 per core.
Per core, for each channel c the 2 batches x 64 features form exactly 128
partition rows of length LENS[c]; pooling runs along the free dim:
  stage 1: m[i]   = max(x[2i], x[2i+1])      (stride-2 tensor_tensor max)
  stage 2: out[p] = max(m[p], m[p+1])        (unit-stride tensor_tensor max)

DMA strategy (all SWDGE via gpsimd — the HWDGE dynamic queues are serial):
  - all 8 channel loads are emitted first with per-channel SBUF tiles (no
    slot reuse -> no WAR deps -> emission never blocks),
  - output NaN tails are written from a persistent SBUF NaN tile, decoupled
    from compute,
  - per-channel stores write only the valid pooled prefix.
Only the valid input prefix is ever read from HBM.
"""

import numpy as np

# ---- problem constants (hardcoded; kernel.py must be self-contained) ----
B, C, F, S = 16, 8, 64, 8192
K, STRIDE = 4, 2
P_OUT = (S - K) // STRIDE + 1  # 4095
LENS = [8192, 4096, 2048, 1024, 8192, 4096, 2048, 1024]
N_CORES = 8
B_LOC = B // N_CORES  # 2 batches per core
MAX_TAIL = max(P_OUT - ((L - K) // STRIDE + 1) for L in LENS)  # 3584

_CACHE = {}


def _build_nc():
    import concourse.bacc as bacc
    import concourse.mybir as mybir
    from concourse.tile import TileContext

    nc = bacc.Bacc("TRN2", debug=False, num_devices=N_CORES)
    x = nc.declare_dram_parameter(
        "x", [B_LOC, C, F, S], mybir.dt.float32, isOutput=False
    )
    out = nc.declare_dram_parameter(
        "out", [B_LOC, C, F, P_OUT], mybir.dt.float32, isOutput=True
    )
    x_ap = x.ap()
    out_ap = out.ap()

    order = [0, 4, 1, 5, 2, 6, 3, 7]  # big channels first

    with TileContext(nc) as tc:
        with tc.tile_pool(name="xin", bufs=1) as in_pool, \
             tc.tile_pool(name="nan", bufs=1) as nan_pool, \
             tc.tile_pool(name="mid", bufs=2) as mid_pool, \
             tc.tile_pool(name="res", bufs=2) as out_pool:

            # 1) all loads up-front, per-channel tiles (no deps)
            xins = {}
            for c in order:
                L = LENS[c]
                xin = in_pool.tile([128, L], mybir.dt.float32, tag=f"xin{c}")
                nc.gpsimd.dma_start(out=xin[:], in_=x_ap[:, c, :, 0:L])
                xins[c] = xin

            # 2) NaN tails of the output, from a persistent constant tile
            nan_tile = nan_pool.tile([128, MAX_TAIL], mybir.dt.float32)
            nc.vector.memset(nan_tile[:], float("nan"))
            for c in order:
                L = LENS[c]
                Pv = (L - K) // STRIDE + 1
                tail = P_OUT - Pv
                if tail:
                    nc.gpsimd.dma_start(
                        out=out_ap[:, c, :, Pv:P_OUT], in_=nan_tile[:, 0:tail]
                    )

            # 3) pool each channel, store valid prefix
            for c in order:
                L = LENS[c]
                Pv = (L - K) // STRIDE + 1
                xin = xins[c]

                m = mid_pool.tile([128, L // 2], mybir.dt.float32, tag="mid")
                x2 = xin[:].rearrange("p (n two) -> p n two", two=2)
                nc.vector.tensor_max(m[:], x2[:, :, 0], x2[:, :, 1])

                o = out_pool.tile([128, Pv], mybir.dt.float32, tag="res")
                nc.vector.tensor_max(o[:], m[:, 0:Pv], m[:, 1:Pv + 1])

                nc.gpsimd.dma_start(out=out_ap[:, c, :, 0:Pv], in_=o[:])
    nc.compile()
    return nc


def _get_nc():
    if "nc" not in _CACHE:
        _CACHE["nc"] = _build_nc()
    return _CACHE["nc"]


def kernel(x: np.ndarray) -> np.ndarray:
    from concourse.bass_utils import run_bass_kernel_spmd

    x = np.asarray(x, dtype=np.float32)
    assert x.shape == (B, C, F, S), x.shape

    nc = _get_nc()
    in_maps = [
        {"x": np.ascontiguousarray(x[i * B_LOC:(i + 1) * B_LOC])}
        for i in range(N_CORES)
    ]
    res = run_bass_kernel_spmd(nc, in_maps, list(range(N_CORES)))
    return np.concatenate([r["out"] for r in res.results], axis=0)


# revision 8
# speedup vs baseline: 2.3806x; 1.2134x over previous
"""MaxPool1d(K=4, stride=2, VALID) over ragged NaN-padded sequences.

Full input:  x  [16, 8, 64, 8192] f32, channel c valid prefix LENS[c], NaN tail.
Full output:    [16, 8, 64, 4095] f32, pooled valid prefix, NaN tail.

Sharding: data-parallel over batch — 16 batches / 8 cores = 2 per core.
Per core, for each channel c the 2 batches x 64 features form exactly 128
partition rows of length LENS[c]; pooling runs along the free dim:
  stage 1: m[i]   = max(x[2i], x[2i+1])      (stride-2 tensor_tensor max)
  stage 2: out[p] = max(m[p], m[p+1])        (unit-stride tensor_tensor max)

DMA strategy (all bulk traffic SWDGE via gpsimd — HWDGE dynamic queues are
serial ~27 GB/s, used only for two dependency-free NaN-tail stores):
  - every transfer is chunked to [128, <=2048] (<=1 MB, 8 KB rows),
  - stage-1 compute runs per loaded chunk so DVE chases the loads,
  - output NaN tails come from a persistent SBUF NaN tile, independent of
    compute,
  - stores write only the valid pooled prefix.
Only the valid input prefix is ever read from HBM.
"""

import numpy as np

# ---- problem constants (hardcoded; kernel.py must be self-contained) ----
B, C, F, S = 16, 8, 64, 8192
K, STRIDE = 4, 2
P_OUT = (S - K) // STRIDE + 1  # 4095
LENS = [8192, 4096, 2048, 1024, 8192, 4096, 2048, 1024]
N_CORES = 8
B_LOC = B // N_CORES  # 2 batches per core
MAX_TAIL = max(P_OUT - ((L - K) // STRIDE + 1) for L in LENS)  # 3584
CK = 2048  # DMA chunk width (columns)

_CACHE = {}


def _chunks(n):
    return [(s, min(CK, n - s)) for s in range(0, n, CK)]


def _build_nc():
    import concourse.bacc as bacc
    import concourse.mybir as mybir
    from concourse.tile import TileContext

    nc = bacc.Bacc("TRN2", debug=False, num_devices=N_CORES)
    x = nc.declare_dram_parameter(
        "x", [B_LOC, C, F, S], mybir.dt.float32, isOutput=False
    )
    out = nc.declare_dram_parameter(
        "out", [B_LOC, C, F, P_OUT], mybir.dt.float32, isOutput=True
    )
    x_ap = x.ap()
    out_ap = out.ap()

    order = [0, 4, 1, 5, 2, 6, 3, 7]  # big channels first

    with TileContext(nc) as tc:
        with tc.tile_pool(name="xin", bufs=8) as in_pool, \
             tc.tile_pool(name="nan", bufs=1) as nan_pool, \
             tc.tile_pool(name="mid", bufs=2) as mid_pool, \
             tc.tile_pool(name="res", bufs=2) as out_pool:

            nan_tile = nan_pool.tile([128, MAX_TAIL], mybir.dt.float32)
            nc.vector.memset(nan_tile[:], float("nan"))

            def emit_nan_tail(c, eng):
                L = LENS[c]
                Pv = (L - K) // STRIDE + 1
                if Pv >= P_OUT:
                    return
                for (s0, w) in _chunks(P_OUT - Pv):
                    eng.dma_start(
                        out=out_ap[:, c, :, Pv + s0:Pv + s0 + w],
                        in_=nan_tile[:, s0:s0 + w],
                    )

            done_nan = set()
            for ci, c in enumerate(order):
                L = LENS[c]
                Pv = (L - K) // STRIDE + 1

                # chunked load + stage-1 per chunk
                m = mid_pool.tile([128, L // 2], mybir.dt.float32, tag="mid")
                for (s0, w) in _chunks(L):
                    xin = in_pool.tile([128, w], mybir.dt.float32, tag="xin")
                    nc.gpsimd.dma_start(out=xin[:], in_=x_ap[:, c, :, s0:s0 + w])
                    x2 = xin[:].rearrange("p (n two) -> p n two", two=2)
                    nc.vector.tensor_max(
                        m[:, s0 // 2:(s0 + w) // 2], x2[:, :, 0], x2[:, :, 1]
                    )

                # after the first two (big) channels' loads are queued, slip
                # the dependency-free NaN-tail fills into the pipeline:
                # c1/c5 tails on the two HWDGE rings, the rest on gpsimd.
                if ci == 1:
                    emit_nan_tail(1, nc.sync)
                    emit_nan_tail(5, nc.scalar)
                    for cc in (2, 6, 3, 7):
                        emit_nan_tail(cc, nc.gpsimd)
                    done_nan.update({1, 5, 2, 6, 3, 7})

                # stage 2 + chunked store of the valid prefix
                o = out_pool.tile([128, Pv], mybir.dt.float32, tag="res")
                nc.vector.tensor_max(o[:], m[:, 0:Pv], m[:, 1:Pv + 1])
                for (s0, w) in _chunks(Pv):
                    nc.gpsimd.dma_start(
                        out=out_ap[:, c, :, s0:s0 + w], in_=o[:, s0:s0 + w]
                    )
    nc.compile()
    return nc


def _get_nc():
    if "nc" not in _CACHE:
        _CACHE["nc"] = _build_nc()
    return _CACHE["nc"]


def kernel(x: np.ndarray) -> np.ndarray:
    from concourse.bass_utils import run_bass_kernel_spmd

    x = np.asarray(x, dtype=np.float32)
    assert x.shape == (B, C, F, S), x.shape

    nc = _get_nc()
    in_maps = [
        {"x": np.ascontiguousarray(x[i * B_LOC:(i + 1) * B_LOC])}
        for i in range(N_CORES)
    ]
    res = run_bass_kernel_spmd(nc, in_maps, list(range(N_CORES)))
    return np.concatenate([r["out"] for r in res.results], axis=0)
